# revision 5
# baseline (speedup 1.0000x reference)
"""Swin-style windowed local self-attention (LN -> QKV -> 7x7 window MHA
with relative position bias -> proj) on 8 Trainium2 NeuronCores.

Sharding: pure data parallel over B*T (24 images -> 3 per core).

v3 design -- engine-balance rework of v2.  Differences:
  - LN stats are computed in BROADCAST form: all-ones [128,128] lhsT
    matmuls give per-token sum / sumsq replicated across all 128
    partitions directly in PSUM, so the per-token mean/var math runs as
    [128, 392] multi-lane ops (no single-lane [1,N] row ops, no gpsimd
    partition broadcasts).
  - scores use 3 PSUM banks of [98, 392] (bank j holds heads {j, 3+j,
    6+j, 9+j}); bias+mask is accumulated first via an identity-weight
    matmul, 3 exp ops per pair instead of 4.
  - softmax normalization is DEFERRED: P = exp(S) is used unnormalized
    in P@V; the per-(head,query) denominators Z are computed head-major
    into a [128, 294] PSUM tile by 12 small ones-lhsT matmuls (rows
    32j = head 4q+j), one reciprocal, and the normalization is folded
    into the P@V PSUM eviction as a single [128, 294] multiply.  This
    removes all big [98, 1176] normalize ops from the critical DVE path.
  - v for all 24 pairs of a group is computed up front (group phase) so
    the per-pair loop only touches score/Z/PV/proj PSUM banks.
  - engine assignment spreads evictions: ACT does exp + qk/av
    evictions, DVE does LN vector math + softmax recip/evict-mult +
    y evictions, GPSIMD does LN scalar folds.
  - phase C is SOFTWARE-PIPELINED: pair p+1's score matmuls are emitted
    before pair p's Z/PV/proj so the in-order PE queue never stalls on
    the exp (ACT); the relative-position bias is applied as a
    precomputed exp(bias) multiply on DVE (exp(s+b) = exp(s)*exp(b),
    exact cross-window masking since exp(-30000) == 0), which removes
    the per-pair bias matmuls from the PE.
"""

import sys

if "/opt/trn_rl_repo" not in sys.path:
    sys.path.insert(0, "/opt/trn_rl_repo")

import numpy as np
import ml_dtypes

import concourse.bacc as bacc
import concourse.bass as bass
import concourse.tile as tile
import concourse.mybir as mybir
from concourse.bass_utils import run_bass_kernel_spmd

F32 = mybir.dt.float32
BF16 = mybir.dt.bfloat16

N_CORES = 8
B, T, H, W, D = 4, 6, 56, 56, 384
WSZ = 7
NH = 12
HD = D // NH            # 32
N = WSZ * WSZ           # 49 tokens / window
IMGS_CORE = (B * T) // N_CORES     # 3
TOK_CORE = IMGS_CORE * H * W       # 9408
NPAIR = TOK_CORE // (2 * N)        # 96 window pairs / core
PT = 2 * N                         # 98 tokens / pair
EPS = 1e-5

N_GROUPS = 4
PAIRS_G = NPAIR // N_GROUPS        # 24
TOK_G = PAIRS_G * PT               # 2352
CH = 4 * PT                        # 392-token chunks (stats + q/k)
NCH_G = TOK_G // CH                # 6 chunks / group
NCH = NPAIR * PT // CH             # 24 chunks / core
MASKVAL = -30000.0

Exp = mybir.ActivationFunctionType.Exp
Sqrt = mybir.ActivationFunctionType.Sqrt
Copy = mybir.ActivationFunctionType.Copy
MUL = mybir.AluOpType.mult
ADD = mybir.AluOpType.add
SUB = mybir.AluOpType.subtract


def _rel_index(w):
    coords = np.stack(np.meshgrid(np.arange(w), np.arange(w), indexing="ij")).reshape(2, -1)
    rel = coords[:, :, None] - coords[:, None, :]
    return (rel[0] + w - 1) * (2 * w - 1) + (rel[1] + w - 1)


def build_program(repeat=1, with_bias=False, n_stats=NCH, n_groups=N_GROUPS,
                  pairs_g=PAIRS_G, stage_upto=9, z_merge=False, qk_evict="act",
                  bias_mode="expb_dve", bufs_a=4, bufs_b=3, bufs_c=3):
    nc = bacc.Bacc("TRN2", target_bir_lowering=False, debug=False, num_devices=N_CORES)

    xT_d = nc.dram_tensor("xT", [D, TOK_CORE], BF16, kind="ExternalInput")
    qkw_d = nc.dram_tensor("qkw", [D, 2 * D], BF16, kind="ExternalInput")
    vw_d = nc.dram_tensor("vw", [D, D], BF16, kind="ExternalInput")
    pw_d = nc.dram_tensor("pw", [D, D], BF16, kind="ExternalInput")
    cqkr_d = nc.dram_tensor("cqkr", [2 * D], BF16, kind="ExternalInput")
    cvr_d = nc.dram_tensor("cvr", [D], BF16, kind="ExternalInput")
    pbr_d = nc.dram_tensor("pbr", [D], BF16, kind="ExternalInput")
    b2_d = nc.dram_tensor("b2", [4, PT, 3 * PT], BF16, kind="ExternalInput")
    expb_d = nc.dram_tensor("expb", [PT, NH * PT], BF16, kind="ExternalInput")
    i98_d = nc.dram_tensor("i98", [PT, PT], BF16, kind="ExternalInput")
    y_d = nc.dram_tensor("y", [TOK_CORE, D], F32, kind="ExternalOutput")

    from contextlib import ExitStack
    with tile.TileContext(nc) as tc, ExitStack() as ctx:
        const = ctx.enter_context(tc.tile_pool(name="const", bufs=1))
        grp = ctx.enter_context(tc.tile_pool(name="grp", bufs=2))
        avp = ctx.enter_context(tc.tile_pool(name="avp", bufs=2))
        wA = ctx.enter_context(tc.tile_pool(name="wA", bufs=bufs_a))
        wB = ctx.enter_context(tc.tile_pool(name="wB", bufs=bufs_b))
        wC = ctx.enter_context(tc.tile_pool(name="wC", bufs=bufs_c))
        ps_sc = ctx.enter_context(tc.tile_pool(name="ps_sc", bufs=4, space="PSUM"))
        ps_zo = ctx.enter_context(tc.tile_pool(name="ps_zo", bufs=2, space="PSUM"))
        ps_gen = ctx.enter_context(tc.tile_pool(name="ps_gen", bufs=2, space="PSUM"))

        # ---- resident constants -------------------------------------------------
        xT = [const.tile([128, TOK_CORE], BF16, name=f"xT{k}", tag=f"xT{k}") for k in range(3)]
        for k in range(3):
            nc.sync.dma_start(out=xT[k][:], in_=xT_d[128 * k:128 * (k + 1), :])
        qkw_sb = [const.tile([128, 2 * D], BF16, name=f"qkw{k}", tag=f"qkw{k}") for k in range(3)]
        vw_sb = [const.tile([128, D], BF16, name=f"vw{k}", tag=f"vw{k}") for k in range(3)]
        pw_sb = [const.tile([128, D], BF16, name=f"pw{k}", tag=f"pw{k}") for k in range(3)]
        for k in range(3):
            nc.sync.dma_start(out=qkw_sb[k][:], in_=qkw_d[128 * k:128 * (k + 1), :])
            nc.sync.dma_start(out=vw_sb[k][:], in_=vw_d[128 * k:128 * (k + 1), :])
            nc.sync.dma_start(out=pw_sb[k][:], in_=pw_d[128 * k:128 * (k + 1), :])
        cqkr_sb = const.tile([1, 2 * D], BF16, name="cqkr", tag="cqkr")
        nc.sync.dma_start(out=cqkr_sb[:], in_=cqkr_d[:])
        cvr_sb = const.tile([1, D], BF16, name="cvr", tag="cvr")
        nc.sync.dma_start(out=cvr_sb[:], in_=cvr_d[:])
        pbr_sb = const.tile([1, D], BF16, name="pbr", tag="pbr")
        nc.sync.dma_start(out=pbr_sb[:], in_=pbr_d[:])
        b2_sb = [const.tile([PT, 3 * PT], BF16, name=f"b2_{j}", tag=f"b2_{j}") for j in range(4)]
        for j in range(4):
            nc.sync.dma_start(out=b2_sb[j][:], in_=b2_d[j])
        i98_sb = const.tile([PT, PT], BF16, name="i98", tag="i98")
        nc.sync.dma_start(out=i98_sb[:], in_=i98_d[:])
        expb_sb = const.tile([PT, NH * PT], BF16, name="expb", tag="expb")
        nc.sync.dma_start(out=expb_sb[:], in_=expb_d[:])
        ones128 = const.tile([128, 128], BF16, name="ones128", tag="ones128")
        nc.vector.memset(ones128[:], 1.0)
        onesz = const.tile([PT, 32], BF16, name="onesz", tag="onesz")
        nc.vector.memset(onesz[:], 1.0)
        onesr_sb = const.tile([1, 512], BF16, name="onesr", tag="onesr")
        nc.vector.memset(onesr_sb[:], 1.0)

        rep_ctx = tc.For_i(0, repeat, 1) if repeat > 1 else None
        if rep_ctx is not None:
            rep_ctx.__enter__()

        # ---- LN stats (broadcast form) + in-place normalize ---------------
        # Emitted PER GROUP (interleaved with phase B/C of the previous
        # group) so the PE-heavy attention work hides the DVE/ACT-heavy
        # LN work instead of serializing behind it.
        def emit_stats(c):
            c0 = CH * c
            zs = ps_sc.tile([128, CH], F32, name="zs", tag="sc")
            zq = ps_sc.tile([128, CH], F32, name="zq", tag="sc")
            sq = [wC.tile([128, CH], BF16, name=f"sq{k}", tag=f"sq{k}") for k in range(3)]
            for k in range(3):
                nc.vector.tensor_tensor(
                    out=sq[k][:], in0=xT[k][:, c0:c0 + CH], in1=xT[k][:, c0:c0 + CH],
                    op=MUL,
                )
                nc.tensor.matmul(zs[:], lhsT=ones128[:], rhs=xT[k][:, c0:c0 + CH],
                                 start=(k == 0), stop=(k == 2), tile_position=(0, 0))
            for k in range(3):
                nc.tensor.matmul(zq[:], lhsT=ones128[:], rhs=sq[k][:],
                                 start=(k == 0), stop=(k == 2), tile_position=(0, 0))
            mu = wC.tile([128, CH], BF16, name="mu", tag="mu")
            nc.scalar.activation(out=mu[:], in_=zs[:], func=Copy, scale=1.0 / D)
            v1 = wC.tile([128, CH], F32, name="v1", tag="v1")
            nc.vector.tensor_scalar(out=v1[:], in0=zq[:], scalar1=1.0 / D, scalar2=EPS,
                                    op0=MUL, op1=ADD)
            m2 = wC.tile([128, CH], F32, name="m2", tag="m2")
            nc.vector.tensor_tensor(out=m2[:], in0=mu[:], in1=mu[:], op=MUL)
            nc.gpsimd.tensor_tensor(out=v1[:], in0=v1[:], in1=m2[:], op=SUB)
            nc.vector.reciprocal(out=v1[:], in_=v1[:])
            isr = wC.tile([128, CH], BF16, name="isr", tag="isr")
            nc.scalar.activation(out=isr[:], in_=v1[:], func=Sqrt)
            for k in range(3):
                nc.gpsimd.tensor_tensor(
                    out=xT[k][:, c0:c0 + CH], in0=xT[k][:, c0:c0 + CH],
                    in1=mu[:], op=SUB,
                )
                nc.vector.tensor_tensor(
                    out=xT[k][:, c0:c0 + CH], in0=xT[k][:, c0:c0 + CH],
                    in1=isr[:], op=MUL,
                )

        for g in range(n_groups if stage_upto >= 1 else 0):
            T0 = TOK_G * g
            for c in range(NCH_G * g, NCH_G * (g + 1)):
                if c < n_stats:
                    emit_stats(c)
            qk = [grp.tile([128, TOK_G], BF16, name=f"qk{m}", tag=f"qk{m}") for m in range(6)]

            # ---- phase B: q/k projections (feature-major) ----------------------
            for cc in range(NCH_G):
                c0 = T0 + CH * cc
                for m in range(6):
                    pq = ps_gen.tile([128, CH], F32, name="pq", tag="gen")
                    if with_bias:
                        nc.tensor.matmul(
                            pq[:], lhsT=cqkr_sb[0:1, 128 * m:128 * (m + 1)],
                            rhs=onesr_sb[0:1, 0:CH],
                            start=True, stop=False, tile_position=(0, 0),
                        )
                    for k in range(3):
                        nc.tensor.matmul(
                            pq[:],
                            lhsT=qkw_sb[k][:, 128 * m:128 * (m + 1)],
                            rhs=xT[k][:, c0:c0 + CH],
                            start=(k == 0 and not with_bias), stop=(k == 2),
                            tile_position=(0, 0),
                        )
                    if qk_evict == "act" or m < 3:
                        nc.scalar.activation(out=qk[m][:, CH * cc:CH * (cc + 1)],
                                             in_=pq[:], func=Copy)
                    else:
                        nc.vector.tensor_copy(out=qk[m][:, CH * cc:CH * (cc + 1)],
                                              in_=pq[:])

            # ---- phase B2: v for all pairs of the group (token-major) ----------
            av_g = avp.tile([PT, pairs_g * D], BF16, name="av", tag="av")
            for p in range(pairs_g):
                t0 = T0 + PT * p
                pv = ps_gen.tile([PT, D], F32, name="pv", tag="gen")
                if with_bias:
                    nc.tensor.matmul(pv[:], lhsT=onesr_sb[0:1, 0:PT], rhs=cvr_sb[:],
                                     start=True, stop=False, tile_position=(0, 0))
                for k in range(3):
                    nc.tensor.matmul(
                        pv[:], lhsT=xT[k][:, t0:t0 + PT], rhs=vw_sb[k][:],
                        start=(k == 0 and not with_bias), stop=(k == 2),
                        tile_position=(0, 0),
                    )
                nc.scalar.activation(out=av_g[:, D * p:D * (p + 1)], in_=pv[:], func=Copy)

            if stage_upto < 2:
                continue
            # ---- phase C: per window pair, SOFTWARE-PIPELINED ------------------
            # The PE executes its queue in order; Z(p) depends on exp(p) (ACT),
            # so emitting Z right after the score matmuls stalls the PE for the
            # whole exp.  Instead emit pair p+1's bias/score matmuls BEFORE
            # pair p's Z/PV/proj so the PE always has exp-independent work.
            use_mm = bias_mode == "mm"
            state = {}

            def emit_front(p):
                qc = PT * p
                sc_j = []
                for j in range(4):
                    sc = ps_sc.tile([PT, 3 * PT], F32, name="sc", tag="sc")
                    if use_mm:
                        nc.tensor.matmul(sc[:], lhsT=i98_sb[:], rhs=b2_sb[j][:],
                                         start=True, stop=False, tile_position=(0, 0))
                    sc_j.append(sc)
                for j in range(4):
                    hb = 32 * j
                    for quad in range(3):
                        nc.tensor.matmul(
                            sc_j[j][:, PT * quad:PT * (quad + 1)],
                            lhsT=qk[3 + quad][hb:hb + 32, qc:qc + PT],
                            rhs=qk[quad][hb:hb + 32, qc:qc + PT],
                            start=(quad == 0 and not use_mm), stop=(quad == 2),
                            tile_position=(hb, 0),
                        )
                if stage_upto < 3:
                    return
                p_t = wA.tile([PT, NH * PT], BF16, name="pt", tag="pt")
                for j in range(4):
                    nc.scalar.activation(
                        out=p_t[:, 3 * PT * j:3 * PT * (j + 1)], in_=sc_j[j][:],
                        func=Exp,
                    )
                if bias_mode == "expb_pool":
                    for hf in range(2):
                        cl = 6 * PT * hf
                        nc.gpsimd.tensor_tensor(
                            out=p_t[:, cl:cl + 6 * PT], in0=p_t[:, cl:cl + 6 * PT],
                            in1=expb_sb[:, cl:cl + 6 * PT], op=MUL,
                        )
                elif bias_mode == "expb_dve":
                    for hf in range(2):
                        cl = 6 * PT * hf
                        nc.vector.tensor_tensor(
                            out=p_t[:, cl:cl + 6 * PT], in0=p_t[:, cl:cl + 6 * PT],
                            in1=expb_sb[:, cl:cl + 6 * PT], op=MUL,
                        )
                elif bias_mode == "expb_split":
                    nc.vector.tensor_tensor(
                        out=p_t[:, 0:6 * PT], in0=p_t[:, 0:6 * PT],
                        in1=expb_sb[:, 0:6 * PT], op=MUL,
                    )
                    nc.gpsimd.tensor_tensor(
                        out=p_t[:, 6 * PT:12 * PT], in0=p_t[:, 6 * PT:12 * PT],
                        in1=expb_sb[:, 6 * PT:12 * PT], op=MUL,
                    )
                state[p] = p_t

            def emit_back(p):
                if stage_upto < 4 or p not in state:
                    return
                p_t = state.pop(p)
                zqp = ps_zo.tile([128, 512], F32, name="zq2", tag="zo")
                if z_merge:
                    for jj in range(4):
                        nc.tensor.matmul(
                            zqp[32 * jj:32 * (jj + 1), 0:3 * PT],
                            lhsT=onesz[:],
                            rhs=p_t[:, 3 * PT * jj:3 * PT * (jj + 1)],
                            start=True, stop=True, tile_position=(0, 32 * jj),
                        )
                else:
                    for quad in range(3):
                        for jj in range(4):
                            h = 4 * quad + jj
                            pcol = 3 * PT * (h % 4) + PT * (h // 4)
                            nc.tensor.matmul(
                                zqp[32 * jj:32 * (jj + 1), PT * quad:PT * (quad + 1)],
                                lhsT=onesz[:],
                                rhs=p_t[:, pcol:pcol + PT],
                                start=True, stop=True, tile_position=(0, 32 * jj),
                            )
                rz = wB.tile([128, 3 * PT], BF16, name="rz", tag="rz")
                with nc.allow_low_precision(reason="1/Z bf16; feeds bf16 multiply"):
                    nc.vector.reciprocal(out=rz[:], in_=zqp[:, 0:3 * PT])

                if stage_upto < 5:
                    return
                ao = ps_zo.tile([128, 512], F32, name="ao", tag="zo")
                for quad in range(3):
                    for jj in range(4):
                        h = 4 * quad + jj
                        pcol = 3 * PT * (h % 4) + PT * (h // 4)
                        nc.tensor.matmul(
                            ao[32 * jj:32 * (jj + 1), PT * quad:PT * (quad + 1)],
                            lhsT=av_g[:, D * p + HD * h:D * p + HD * (h + 1)],
                            rhs=p_t[:, pcol:pcol + PT],
                            start=True, stop=True, tile_position=(0, 32 * jj),
                        )
                atT = wB.tile([128, 3 * PT], BF16, name="atT", tag="atT")
                nc.vector.tensor_tensor(out=atT[:], in0=ao[:, 0:3 * PT], in1=rz[:], op=MUL)

                if stage_upto < 6:
                    return
                pp = ps_gen.tile([PT, D], F32, name="pp", tag="gen")
                if with_bias:
                    nc.tensor.matmul(pp[:], lhsT=onesr_sb[0:1, 0:PT], rhs=pbr_sb[:],
                                     start=True, stop=False, tile_position=(0, 0))
                for quad in range(3):
                    nc.tensor.matmul(
                        pp[:], lhsT=atT[:, PT * quad:PT * (quad + 1)], rhs=pw_sb[quad][:],
                        start=(quad == 0 and not with_bias), stop=(quad == 2),
                        tile_position=(0, 0),
                    )
                y_sb = wC.tile([PT, D], F32, name="y", tag="y")
                nc.vector.tensor_copy(out=y_sb[:], in_=pp[:])
                r0 = T0 + PT * p
                nc.sync.dma_start(out=y_d[r0:r0 + PT, :], in_=y_sb[:])

            for p in range(pairs_g):
                emit_front(p)
                if p > 0:
                    emit_back(p - 1)
            emit_back(pairs_g - 1)

        if rep_ctx is not None:
            rep_ctx.__exit__(None, None, None)

    nc.compile()
    return nc


_NC_CACHE = {}


def _get_program(with_bias=False):
    key = ("nc", with_bias)
    if key not in _NC_CACHE:
        _NC_CACHE[key] = build_program(with_bias=with_bias)
    return _NC_CACHE[key]


def _window_order(xf):
    BT = xf.shape[0]
    x6 = xf.reshape(BT, H // WSZ, WSZ, W // WSZ, WSZ, D)
    return np.ascontiguousarray(x6.transpose(0, 1, 3, 2, 4, 5)).reshape(-1, D)


def _window_unorder(yw):
    BT = B * T
    y6 = yw.reshape(BT, H // WSZ, W // WSZ, WSZ, WSZ, D)
    return np.ascontiguousarray(y6.transpose(0, 1, 3, 2, 4, 5)).reshape(BT, H, W, D)


def prepare_inputs(x, ln_g, ln_b, qkv_w, qkv_b, proj_w, proj_b, rel_bias_table):
    x = np.asarray(x, np.float32)
    ln_g = np.asarray(ln_g, np.float32)
    ln_b = np.asarray(ln_b, np.float32)
    qkv_w = np.asarray(qkv_w, np.float32)
    qkv_b = np.asarray(qkv_b, np.float32)
    proj_w = np.asarray(proj_w, np.float32)
    proj_b = np.asarray(proj_b, np.float32)
    rel_bias_table = np.asarray(rel_bias_table, np.float32)

    scale = HD ** -0.5
    wq = qkv_w[:, :D] * ln_g[:, None] * scale
    wk = qkv_w[:, D:2 * D] * ln_g[:, None]
    wv = qkv_w[:, 2 * D:] * ln_g[:, None]
    cq = (ln_b @ qkv_w[:, :D] + qkv_b[:D]) * scale
    ck = ln_b @ qkv_w[:, D:2 * D] + qkv_b[D:2 * D]
    cv = ln_b @ qkv_w[:, 2 * D:] + qkv_b[2 * D:]

    qkw = np.concatenate([wq, wk], axis=1).astype(ml_dtypes.bfloat16)
    cqk = np.concatenate([cq, ck]).astype(np.float32)

    idx = _rel_index(WSZ)
    bias = rel_bias_table[idx.reshape(-1)].reshape(N, N, NH)  # [q, k, h]
    # strip-major bias tables: table j holds heads {j, 4+j, 8+j}
    b2 = np.full((4, PT, 3 * PT), MASKVAL, np.float32)
    for j in range(4):
        for quad in range(3):
            h = 4 * quad + j
            blkT = bias[:, :, h].T  # [k, q]
            for w in range(2):
                b2[j, N * w:N * (w + 1), PT * quad + N * w:PT * quad + N * (w + 1)] = blkT

    xw = _window_order(x.reshape(B * T, H, W, D))

    common = {
        "qkw": qkw,
        "vw": wv.astype(ml_dtypes.bfloat16),
        "pw": proj_w.astype(ml_dtypes.bfloat16),
        "cqkr": cqk.astype(ml_dtypes.bfloat16),
        "cvr": cv.astype(ml_dtypes.bfloat16),
        "pbr": proj_b.astype(ml_dtypes.bfloat16),
        "b2": b2.astype(ml_dtypes.bfloat16),
        "expb": np.exp(np.concatenate([b2[j] for j in range(4)], axis=1)).astype(ml_dtypes.bfloat16),
        "i98": np.eye(PT, dtype=np.float32).astype(ml_dtypes.bfloat16),
    }
    in_maps = []
    for c in range(N_CORES):
        m = dict(common)
        xc = xw[TOK_CORE * c:TOK_CORE * (c + 1)]
        m["xT"] = np.ascontiguousarray(xc.T).astype(ml_dtypes.bfloat16)
        in_maps.append(m)
    return in_maps


def kernel(x, ln_g, ln_b, qkv_w, qkv_b, proj_w, proj_b, rel_bias_table):
    in_maps = prepare_inputs(x, ln_g, ln_b, qkv_w, qkv_b, proj_w, proj_b, rel_bias_table)
    with_bias = any(
        np.any(np.asarray(in_maps[0][k], np.float32) != 0.0)
        for k in ("cqkr", "cvr", "pbr")
    )
    nc = _get_program(with_bias=with_bias)
    res = run_bass_kernel_spmd(nc, in_maps, core_ids=list(range(N_CORES)))
    yw = np.concatenate([res.results[c]["y"] for c in range(N_CORES)], axis=0)
    out = _window_unorder(yw).reshape(B, T, H, W, D)
    return out.astype(np.float32)


# revision 6
# speedup vs baseline: 1.2395x; 1.2395x over previous
"""Swin-style windowed local self-attention (LN -> QKV -> 7x7 window MHA
with relative position bias -> proj) on 8 Trainium2 NeuronCores.

Sharding: pure data parallel over B*T (24 images -> 3 per core).

v3 design -- engine-balance rework of v2.  Differences:
  - LN stats are computed in BROADCAST form: all-ones [128,128] lhsT
    matmuls give per-token sum / sumsq replicated across all 128
    partitions directly in PSUM, so the per-token mean/var math runs as
    [128, 392] multi-lane ops (no single-lane [1,N] row ops, no gpsimd
    partition broadcasts).
  - scores use 3 PSUM banks of [98, 392] (bank j holds heads {j, 3+j,
    6+j, 9+j}); bias+mask is accumulated first via an identity-weight
    matmul, 3 exp ops per pair instead of 4.
  - softmax normalization is DEFERRED: P = exp(S) is used unnormalized
    in P@V; the per-(head,query) denominators Z are computed head-major
    into a [128, 294] PSUM tile by 12 small ones-lhsT matmuls (rows
    32j = head 4q+j), one reciprocal, and the normalization is folded
    into the P@V PSUM eviction as a single [128, 294] multiply.  This
    removes all big [98, 1176] normalize ops from the critical DVE path.
  - v for all 24 pairs of a group is computed up front (group phase) so
    the per-pair loop only touches score/Z/PV/proj PSUM banks.
  - engine assignment spreads evictions: ACT does exp + qk/av
    evictions, DVE does LN vector math + softmax recip/evict-mult +
    y evictions, GPSIMD does LN scalar folds.
  - phase C is SOFTWARE-PIPELINED: pair p+1's score matmuls are emitted
    before pair p's Z/PV/proj so the in-order PE queue never stalls on
    the exp (ACT); the relative-position bias is applied as a
    precomputed exp(bias) multiply on DVE (exp(s+b) = exp(s)*exp(b),
    exact cross-window masking since exp(-30000) == 0), which removes
    the per-pair bias matmuls from the PE.
"""

import sys

if "/opt/trn_rl_repo" not in sys.path:
    sys.path.insert(0, "/opt/trn_rl_repo")

import numpy as np
import ml_dtypes

import concourse.bacc as bacc
import concourse.bass as bass
import concourse.tile as tile
import concourse.mybir as mybir
from concourse.bass_utils import run_bass_kernel_spmd

F32 = mybir.dt.float32
BF16 = mybir.dt.bfloat16

N_CORES = 8
B, T, H, W, D = 4, 6, 56, 56, 384
WSZ = 7
NH = 12
HD = D // NH            # 32
N = WSZ * WSZ           # 49 tokens / window
IMGS_CORE = (B * T) // N_CORES     # 3
TOK_CORE = IMGS_CORE * H * W       # 9408
NPAIR = TOK_CORE // (2 * N)        # 96 window pairs / core
PT = 2 * N                         # 98 tokens / pair
EPS = 1e-5

N_GROUPS = 4
PAIRS_G = NPAIR // N_GROUPS        # 24
TOK_G = PAIRS_G * PT               # 2352
CH = 4 * PT                        # 392-token chunks (stats + q/k)
NCH_G = TOK_G // CH                # 6 chunks / group
NCH = NPAIR * PT // CH             # 24 chunks / core
MASKVAL = -30000.0

Exp = mybir.ActivationFunctionType.Exp
Sqrt = mybir.ActivationFunctionType.Sqrt
Copy = mybir.ActivationFunctionType.Copy
MUL = mybir.AluOpType.mult
ADD = mybir.AluOpType.add
SUB = mybir.AluOpType.subtract


def _rel_index(w):
    coords = np.stack(np.meshgrid(np.arange(w), np.arange(w), indexing="ij")).reshape(2, -1)
    rel = coords[:, :, None] - coords[:, None, :]
    return (rel[0] + w - 1) * (2 * w - 1) + (rel[1] + w - 1)


def build_program(repeat=1, with_bias=False, n_stats=NCH, n_groups=N_GROUPS,
                  pairs_g=PAIRS_G, stage_upto=9, z_merge=False, qk_evict="act",
                  bias_mode="expb_dve", bufs_a=4, bufs_b=3, bufs_c=3, stagger=2,
                  mult1=True):
    nc = bacc.Bacc("TRN2", target_bir_lowering=False, debug=False, num_devices=N_CORES)

    xT_d = nc.dram_tensor("xT", [D, TOK_CORE], BF16, kind="ExternalInput")
    qkw_d = nc.dram_tensor("qkw", [D, 2 * D], BF16, kind="ExternalInput")
    vw_d = nc.dram_tensor("vw", [D, D], BF16, kind="ExternalInput")
    pw_d = nc.dram_tensor("pw", [D, D], BF16, kind="ExternalInput")
    cqkr_d = nc.dram_tensor("cqkr", [2 * D], BF16, kind="ExternalInput")
    cvr_d = nc.dram_tensor("cvr", [D], BF16, kind="ExternalInput")
    pbr_d = nc.dram_tensor("pbr", [D], BF16, kind="ExternalInput")
    b2_d = nc.dram_tensor("b2", [4, PT, 3 * PT], BF16, kind="ExternalInput")
    expb_d = nc.dram_tensor("expb", [PT, NH * PT], BF16, kind="ExternalInput")
    i98_d = nc.dram_tensor("i98", [PT, PT], BF16, kind="ExternalInput")
    y_d = nc.dram_tensor("y", [TOK_CORE, D], F32, kind="ExternalOutput")

    from contextlib import ExitStack
    with tile.TileContext(nc) as tc, ExitStack() as ctx:
        const = ctx.enter_context(tc.tile_pool(name="const", bufs=1))
        grp = ctx.enter_context(tc.tile_pool(name="grp", bufs=2))
        avp = ctx.enter_context(tc.tile_pool(name="avp", bufs=2))
        wA = ctx.enter_context(tc.tile_pool(name="wA", bufs=bufs_a))
        wB = ctx.enter_context(tc.tile_pool(name="wB", bufs=bufs_b))
        wC = ctx.enter_context(tc.tile_pool(name="wC", bufs=bufs_c))
        ps_sc = ctx.enter_context(tc.tile_pool(name="ps_sc", bufs=4, space="PSUM"))
        ps_zo = ctx.enter_context(tc.tile_pool(name="ps_zo", bufs=2, space="PSUM"))
        ps_gen = ctx.enter_context(tc.tile_pool(name="ps_gen", bufs=2, space="PSUM"))

        # ---- resident constants -------------------------------------------------
        xT = [const.tile([128, TOK_CORE], BF16, name=f"xT{k}", tag=f"xT{k}") for k in range(3)]
        for k in range(3):
            nc.sync.dma_start(out=xT[k][:], in_=xT_d[128 * k:128 * (k + 1), :])
        qkw_sb = [const.tile([128, 2 * D], BF16, name=f"qkw{k}", tag=f"qkw{k}") for k in range(3)]
        vw_sb = [const.tile([128, D], BF16, name=f"vw{k}", tag=f"vw{k}") for k in range(3)]
        pw_sb = [const.tile([128, D], BF16, name=f"pw{k}", tag=f"pw{k}") for k in range(3)]
        for k in range(3):
            nc.sync.dma_start(out=qkw_sb[k][:], in_=qkw_d[128 * k:128 * (k + 1), :])
            nc.sync.dma_start(out=vw_sb[k][:], in_=vw_d[128 * k:128 * (k + 1), :])
            nc.sync.dma_start(out=pw_sb[k][:], in_=pw_d[128 * k:128 * (k + 1), :])
        cqkr_sb = const.tile([1, 2 * D], BF16, name="cqkr", tag="cqkr")
        nc.sync.dma_start(out=cqkr_sb[:], in_=cqkr_d[:])
        cvr_sb = const.tile([1, D], BF16, name="cvr", tag="cvr")
        nc.sync.dma_start(out=cvr_sb[:], in_=cvr_d[:])
        pbr_sb = const.tile([1, D], BF16, name="pbr", tag="pbr")
        nc.sync.dma_start(out=pbr_sb[:], in_=pbr_d[:])
        b2_sb = [const.tile([PT, 3 * PT], BF16, name=f"b2_{j}", tag=f"b2_{j}") for j in range(4)]
        for j in range(4):
            nc.sync.dma_start(out=b2_sb[j][:], in_=b2_d[j])
        i98_sb = const.tile([PT, PT], BF16, name="i98", tag="i98")
        nc.sync.dma_start(out=i98_sb[:], in_=i98_d[:])
        expb_sb = const.tile([PT, NH * PT], BF16, name="expb", tag="expb")
        nc.sync.dma_start(out=expb_sb[:], in_=expb_d[:])
        ones128 = const.tile([128, 128], BF16, name="ones128", tag="ones128")
        nc.vector.memset(ones128[:], 1.0)
        onesz = const.tile([PT, 32], BF16, name="onesz", tag="onesz")
        nc.vector.memset(onesz[:], 1.0)
        onesr_sb = const.tile([1, 512], BF16, name="onesr", tag="onesr")
        nc.vector.memset(onesr_sb[:], 1.0)

        rep_ctx = tc.For_i(0, repeat, 1) if repeat > 1 else None
        if rep_ctx is not None:
            rep_ctx.__enter__()

        # ---- LN stats (broadcast form) + in-place normalize ---------------
        # Emitted PER GROUP (interleaved with phase B/C of the previous
        # group) so the PE-heavy attention work hides the DVE/ACT-heavy
        # LN work instead of serializing behind it.
        def emit_stats(c):
            c0 = CH * c
            zs = ps_sc.tile([128, CH], F32, name="zs", tag="sc")
            zq = ps_sc.tile([128, CH], F32, name="zq", tag="sc")
            sq = [wC.tile([128, CH], BF16, name=f"sq{k}", tag=f"sq{k}") for k in range(3)]
            for k in range(3):
                nc.vector.tensor_tensor(
                    out=sq[k][:], in0=xT[k][:, c0:c0 + CH], in1=xT[k][:, c0:c0 + CH],
                    op=MUL,
                )
                nc.tensor.matmul(zs[:], lhsT=ones128[:], rhs=xT[k][:, c0:c0 + CH],
                                 start=(k == 0), stop=(k == 2), tile_position=(0, 0))
            for k in range(3):
                nc.tensor.matmul(zq[:], lhsT=ones128[:], rhs=sq[k][:],
                                 start=(k == 0), stop=(k == 2), tile_position=(0, 0))
            mu = wC.tile([128, CH], BF16, name="mu", tag="mu")
            nc.scalar.activation(out=mu[:], in_=zs[:], func=Copy, scale=1.0 / D)
            v1 = wC.tile([128, CH], F32, name="v1", tag="v1")
            nc.vector.tensor_scalar(out=v1[:], in0=zq[:], scalar1=1.0 / D, scalar2=EPS,
                                    op0=MUL, op1=ADD)
            m2 = wC.tile([128, CH], F32, name="m2", tag="m2")
            nc.vector.tensor_tensor(out=m2[:], in0=mu[:], in1=mu[:], op=MUL)
            nc.gpsimd.tensor_tensor(out=v1[:], in0=v1[:], in1=m2[:], op=SUB)
            nc.vector.reciprocal(out=v1[:], in_=v1[:])
            isr = wC.tile([128, CH], BF16, name="isr", tag="isr")
            nc.scalar.activation(out=isr[:], in_=v1[:], func=Sqrt)
            for k in range(3):
                nc.gpsimd.tensor_tensor(
                    out=xT[k][:, c0:c0 + CH], in0=xT[k][:, c0:c0 + CH],
                    in1=mu[:], op=SUB,
                )
                nc.vector.tensor_tensor(
                    out=xT[k][:, c0:c0 + CH], in0=xT[k][:, c0:c0 + CH],
                    in1=isr[:], op=MUL,
                )

        for g in range(n_groups if stage_upto >= 1 else 0):
            T0 = TOK_G * g
            for c in range(NCH_G * g, NCH_G * (g + 1)):
                if c < n_stats:
                    emit_stats(c)
            qk = [grp.tile([128, TOK_G], BF16, name=f"qk{m}", tag=f"qk{m}") for m in range(6)]

            # ---- phase B: q/k projections (feature-major) ----------------------
            for cc in range(NCH_G):
                c0 = T0 + CH * cc
                for m in range(6):
                    pq = ps_gen.tile([128, CH], F32, name="pq", tag="gen")
                    if with_bias:
                        nc.tensor.matmul(
                            pq[:], lhsT=cqkr_sb[0:1, 128 * m:128 * (m + 1)],
                            rhs=onesr_sb[0:1, 0:CH],
                            start=True, stop=False, tile_position=(0, 0),
                        )
                    for k in range(3):
                        nc.tensor.matmul(
                            pq[:],
                            lhsT=qkw_sb[k][:, 128 * m:128 * (m + 1)],
                            rhs=xT[k][:, c0:c0 + CH],
                            start=(k == 0 and not with_bias), stop=(k == 2),
                            tile_position=(0, 0),
                        )
                    if qk_evict == "act" or m < 3:
                        nc.scalar.activation(out=qk[m][:, CH * cc:CH * (cc + 1)],
                                             in_=pq[:], func=Copy)
                    else:
                        nc.vector.tensor_copy(out=qk[m][:, CH * cc:CH * (cc + 1)],
                                              in_=pq[:])

            # ---- phase B2: v for all pairs of the group (token-major) ----------
            av_g = avp.tile([PT, pairs_g * D], BF16, name="av", tag="av")
            for p in range(pairs_g):
                t0 = T0 + PT * p
                pv = ps_gen.tile([PT, D], F32, name="pv", tag="gen")
                if with_bias:
                    nc.tensor.matmul(pv[:], lhsT=onesr_sb[0:1, 0:PT], rhs=cvr_sb[:],
                                     start=True, stop=False, tile_position=(0, 0))
                for k in range(3):
                    nc.tensor.matmul(
                        pv[:], lhsT=xT[k][:, t0:t0 + PT], rhs=vw_sb[k][:],
                        start=(k == 0 and not with_bias), stop=(k == 2),
                        tile_position=(0, 0),
                    )
                nc.scalar.activation(out=av_g[:, D * p:D * (p + 1)], in_=pv[:], func=Copy)

            if stage_upto < 2:
                continue
            # ---- phase C: per window pair, SOFTWARE-PIPELINED ------------------
            # The PE executes its queue in order; Z(p) depends on exp(p) (ACT),
            # so emitting Z right after the score matmuls stalls the PE for the
            # whole exp.  Instead emit pair p+1's bias/score matmuls BEFORE
            # pair p's Z/PV/proj so the PE always has exp-independent work.
            use_mm = bias_mode == "mm"
            state = {}

            def emit_front(p):
                qc = PT * p
                sc_j = []
                for j in range(4):
                    sc = ps_sc.tile([PT, 3 * PT], F32, name="sc", tag="sc")
                    if use_mm:
                        nc.tensor.matmul(sc[:], lhsT=i98_sb[:], rhs=b2_sb[j][:],
                                         start=True, stop=False, tile_position=(0, 0))
                    sc_j.append(sc)
                for j in range(4):
                    hb = 32 * j
                    for quad in range(3):
                        nc.tensor.matmul(
                            sc_j[j][:, PT * quad:PT * (quad + 1)],
                            lhsT=qk[3 + quad][hb:hb + 32, qc:qc + PT],
                            rhs=qk[quad][hb:hb + 32, qc:qc + PT],
                            start=(quad == 0 and not use_mm), stop=(quad == 2),
                            tile_position=(hb, 0),
                        )
                if stage_upto < 3:
                    return
                p_t = wA.tile([PT, NH * PT], BF16, name="pt", tag="pt")
                for j in range(4):
                    nc.scalar.activation(
                        out=p_t[:, 3 * PT * j:3 * PT * (j + 1)], in_=sc_j[j][:],
                        func=Exp,
                    )
                if bias_mode == "expb_pool":
                    for hf in range(2):
                        cl = 6 * PT * hf
                        nc.gpsimd.tensor_tensor(
                            out=p_t[:, cl:cl + 6 * PT], in0=p_t[:, cl:cl + 6 * PT],
                            in1=expb_sb[:, cl:cl + 6 * PT], op=MUL,
                        )
                elif bias_mode == "expb_dve":
                    if mult1:
                        nc.vector.tensor_tensor(
                            out=p_t[:], in0=p_t[:], in1=expb_sb[:], op=MUL,
                        )
                    else:
                        for hf in range(2):
                            cl = 6 * PT * hf
                            nc.vector.tensor_tensor(
                                out=p_t[:, cl:cl + 6 * PT], in0=p_t[:, cl:cl + 6 * PT],
                                in1=expb_sb[:, cl:cl + 6 * PT], op=MUL,
                            )
                elif bias_mode == "expb_split":
                    nc.vector.tensor_tensor(
                        out=p_t[:, 0:6 * PT], in0=p_t[:, 0:6 * PT],
                        in1=expb_sb[:, 0:6 * PT], op=MUL,
                    )
                    nc.gpsimd.tensor_tensor(
                        out=p_t[:, 6 * PT:12 * PT], in0=p_t[:, 6 * PT:12 * PT],
                        in1=expb_sb[:, 6 * PT:12 * PT], op=MUL,
                    )
                state[p] = p_t

            def emit_back(p):
                if stage_upto < 4 or p not in state:
                    return
                p_t = state.pop(p)
                zqp = ps_zo.tile([128, 512], F32, name="zq2", tag="zo")
                if z_merge:
                    for jj in range(4):
                        nc.tensor.matmul(
                            zqp[32 * jj:32 * (jj + 1), 0:3 * PT],
                            lhsT=onesz[:],
                            rhs=p_t[:, 3 * PT * jj:3 * PT * (jj + 1)],
                            start=True, stop=True, tile_position=(0, 32 * jj),
                        )
                else:
                    for quad in range(3):
                        for jj in range(4):
                            h = 4 * quad + jj
                            pcol = 3 * PT * (h % 4) + PT * (h // 4)
                            nc.tensor.matmul(
                                zqp[32 * jj:32 * (jj + 1), PT * quad:PT * (quad + 1)],
                                lhsT=onesz[:],
                                rhs=p_t[:, pcol:pcol + PT],
                                start=True, stop=True, tile_position=(0, 32 * jj),
                            )
                rz = wB.tile([128, 3 * PT], BF16, name="rz", tag="rz")
                with nc.allow_low_precision(reason="1/Z bf16; feeds bf16 multiply"):
                    nc.vector.reciprocal(out=rz[:], in_=zqp[:, 0:3 * PT])

                if stage_upto < 5:
                    return
                ao = ps_zo.tile([128, 512], F32, name="ao", tag="zo")
                for quad in range(3):
                    for jj in range(4):
                        h = 4 * quad + jj
                        pcol = 3 * PT * (h % 4) + PT * (h // 4)
                        nc.tensor.matmul(
                            ao[32 * jj:32 * (jj + 1), PT * quad:PT * (quad + 1)],
                            lhsT=av_g[:, D * p + HD * h:D * p + HD * (h + 1)],
                            rhs=p_t[:, pcol:pcol + PT],
                            start=True, stop=True, tile_position=(0, 32 * jj),
                        )
                atT = wB.tile([128, 3 * PT], BF16, name="atT", tag="atT")
                nc.vector.tensor_tensor(out=atT[:], in0=ao[:, 0:3 * PT], in1=rz[:], op=MUL)

                if stage_upto < 6:
                    return
                pp = ps_gen.tile([PT, D], F32, name="pp", tag="gen")
                if with_bias:
                    nc.tensor.matmul(pp[:], lhsT=onesr_sb[0:1, 0:PT], rhs=pbr_sb[:],
                                     start=True, stop=False, tile_position=(0, 0))
                for quad in range(3):
                    nc.tensor.matmul(
                        pp[:], lhsT=atT[:, PT * quad:PT * (quad + 1)], rhs=pw_sb[quad][:],
                        start=(quad == 0 and not with_bias), stop=(quad == 2),
                        tile_position=(0, 0),
                    )
                y_sb = wC.tile([PT, D], F32, name="y", tag="y")
                nc.vector.tensor_copy(out=y_sb[:], in_=pp[:])
                r0 = T0 + PT * p
                nc.sync.dma_start(out=y_d[r0:r0 + PT, :], in_=y_sb[:])

            for p in range(pairs_g):
                emit_front(p)
                if p >= stagger:
                    emit_back(p - stagger)
            for p in range(pairs_g - stagger, pairs_g):
                emit_back(p)

        if rep_ctx is not None:
            rep_ctx.__exit__(None, None, None)

    nc.compile()
    return nc


_NC_CACHE = {}


def _get_program(with_bias=False):
    key = ("nc", with_bias)
    if key not in _NC_CACHE:
        _NC_CACHE[key] = build_program(with_bias=with_bias)
    return _NC_CACHE[key]


def _window_order(xf):
    BT = xf.shape[0]
    x6 = xf.reshape(BT, H // WSZ, WSZ, W // WSZ, WSZ, D)
    return np.ascontiguousarray(x6.transpose(0, 1, 3, 2, 4, 5)).reshape(-1, D)


def _window_unorder(yw):
    BT = B * T
    y6 = yw.reshape(BT, H // WSZ, W // WSZ, WSZ, WSZ, D)
    return np.ascontiguousarray(y6.transpose(0, 1, 3, 2, 4, 5)).reshape(BT, H, W, D)


def prepare_inputs(x, ln_g, ln_b, qkv_w, qkv_b, proj_w, proj_b, rel_bias_table):
    x = np.asarray(x, np.float32)
    ln_g = np.asarray(ln_g, np.float32)
    ln_b = np.asarray(ln_b, np.float32)
    qkv_w = np.asarray(qkv_w, np.float32)
    qkv_b = np.asarray(qkv_b, np.float32)
    proj_w = np.asarray(proj_w, np.float32)
    proj_b = np.asarray(proj_b, np.float32)
    rel_bias_table = np.asarray(rel_bias_table, np.float32)

    scale = HD ** -0.5
    wq = qkv_w[:, :D] * ln_g[:, None] * scale
    wk = qkv_w[:, D:2 * D] * ln_g[:, None]
    wv = qkv_w[:, 2 * D:] * ln_g[:, None]
    cq = (ln_b @ qkv_w[:, :D] + qkv_b[:D]) * scale
    ck = ln_b @ qkv_w[:, D:2 * D] + qkv_b[D:2 * D]
    cv = ln_b @ qkv_w[:, 2 * D:] + qkv_b[2 * D:]

    qkw = np.concatenate([wq, wk], axis=1).astype(ml_dtypes.bfloat16)
    cqk = np.concatenate([cq, ck]).astype(np.float32)

    idx = _rel_index(WSZ)
    bias = rel_bias_table[idx.reshape(-1)].reshape(N, N, NH)  # [q, k, h]
    # strip-major bias tables: table j holds heads {j, 4+j, 8+j}
    b2 = np.full((4, PT, 3 * PT), MASKVAL, np.float32)
    for j in range(4):
        for quad in range(3):
            h = 4 * quad + j
            blkT = bias[:, :, h].T  # [k, q]
            for w in range(2):
                b2[j, N * w:N * (w + 1), PT * quad + N * w:PT * quad + N * (w + 1)] = blkT

    xw = _window_order(x.reshape(B * T, H, W, D))

    common = {
        "qkw": qkw,
        "vw": wv.astype(ml_dtypes.bfloat16),
        "pw": proj_w.astype(ml_dtypes.bfloat16),
        "cqkr": cqk.astype(ml_dtypes.bfloat16),
        "cvr": cv.astype(ml_dtypes.bfloat16),
        "pbr": proj_b.astype(ml_dtypes.bfloat16),
        "b2": b2.astype(ml_dtypes.bfloat16),
        "expb": np.exp(np.concatenate([b2[j] for j in range(4)], axis=1)).astype(ml_dtypes.bfloat16),
        "i98": np.eye(PT, dtype=np.float32).astype(ml_dtypes.bfloat16),
    }
    in_maps = []
    for c in range(N_CORES):
        m = dict(common)
        xc = xw[TOK_CORE * c:TOK_CORE * (c + 1)]
        m["xT"] = np.ascontiguousarray(xc.T).astype(ml_dtypes.bfloat16)
        in_maps.append(m)
    return in_maps


def kernel(x, ln_g, ln_b, qkv_w, qkv_b, proj_w, proj_b, rel_bias_table):
    in_maps = prepare_inputs(x, ln_g, ln_b, qkv_w, qkv_b, proj_w, proj_b, rel_bias_table)
    with_bias = any(
        np.any(np.asarray(in_maps[0][k], np.float32) != 0.0)
        for k in ("cqkr", "cvr", "pbr")
    )
    nc = _get_program(with_bias=with_bias)
    res = run_bass_kernel_spmd(nc, in_maps, core_ids=list(range(N_CORES)))
    yw = np.concatenate([res.results[c]["y"] for c in range(N_CORES)], axis=0)
    out = _window_unorder(yw).reshape(B, T, H, W, D)
    return out.astype(np.float32)
